# revision 9
# baseline (speedup 1.0000x reference)
# Multi-head attention + output projection kernel for 8 TRN2 NeuronCores.
#
# Problem: q,k,v [4,16,2048,64] fp32; w_out [64,64]; b_out [64]
#   out = softmax(q @ k^T / sqrt(64)) @ v @ w_out^T + b_out
#
# Strategy:
#  - 64 (batch, head) pairs sharded 8-per-core (pure data parallel, no collectives).
#  - Host-side layout prep: Q^T duplicated on both partition halves, K^T packed so
#    chunk c and c+8 sit in partition rows 0-63 / 64-127 (fp16, d on partitions),
#    V with a ones-column appended (softmax denominator rides the PE matmul),
#    w_out^T padded to [65,65] with a 1 in the corner (row-sum rides the projection).
#  - Scores are computed transposed [k_chunk, q] with the contract dim d=64 split
#    across PE row groups: two chunk matmuls run concurrently via tile_position
#    row packing, doubling QK^T throughput.
#  - exp on ScalarE straight out of PSUM (no max-subtraction: |scores/8| < ~7,
#    fp16-safe), writing the attn^T tile fp16 to SBUF.
#  - y_ext^T = V_ext^T @ attn^T accumulated in PSUM over k chunks; projection
#    returns to [q, 65] orientation; normalize+bias are per-partition VectorE ops.

import math

import numpy as np

import concourse.bass as bass
import concourse.mybir as mybir
import concourse.tile as tile
from concourse import bacc
from concourse.bass_utils import run_bass_kernel_spmd

F16 = mybir.dt.float16
F32 = mybir.dt.float32

B, H, S, D = 4, 16, 2048, 64
N_CORES = 8
N_HEADS = B * H                    # 64
HPC = N_HEADS // N_CORES           # 8 heads per core
SCALE = 1.0 / math.sqrt(D)         # 1/8

# fp16 Schraudolph exp for the VectorE path: exp(s/8) ~= bitcast_f16(
# int16_trunc(s * EXPA + EXPB)). Constant calibrated for truncation
# (DVE fp32->int16 converts toward zero); max rel err ~3.0%, which the
# softmax normalization and 2048-term averaging wash down to ~6e-3 end-to-end.
EXPA = 1024.0 / (8.0 * math.log(2.0))
EXPB = 15315.75

# test.py can flip these before calling kernel() to capture a profile.
TRACE = False
TRACE_KWARGS = {}
LAST_RESULT = None

_CACHED = {}


def build_bass(hpc=HPC, seq=S, dve_slots=(0, 3, 6)):
    """Build the per-core Bass program. Parameterized so a small config can be
    simulated in CoreSim. Requires seq % 256 == 0.

    dve_slots: which unit indices (mod 8) compute exp on VectorE via the
    Schraudolph approximation instead of ScalarE, balancing the two engines."""
    QB = min(1024, seq)            # q columns per y-accumulation half
    n_m = seq // QB                # y halves per head
    n_ch = seq // 128              # k chunks per head
    half = n_ch // 2               # chunk pairs per head
    n_jj = QB // 512               # 512-wide q blocks per half
    n_qt = QB // 128               # 128-row q tiles per half

    nc = bacc.Bacc("TRN2", target_bir_lowering=False, debug=False)

    qt_d = nc.dram_tensor("qt", [hpc, 128, seq], F16, kind="ExternalInput").ap()
    kt_d = nc.dram_tensor("kt", [hpc, 128, half * 128], F16, kind="ExternalInput").ap()
    vx_d = nc.dram_tensor("vx", [hpc, n_ch, 128, 65], F16, kind="ExternalInput").ap()
    wx_d = nc.dram_tensor("wx", [65, 65], F16, kind="ExternalInput").ap()
    bb_d = nc.dram_tensor("bb", [128, 64], F32, kind="ExternalInput").ap()
    out_d = nc.dram_tensor("out", [hpc, seq, 64], F32, kind="ExternalOutput").ap()

    with tile.TileContext(nc) as tc:
        with (
            tc.tile_pool(name="const", bufs=1) as const_pool,
            tc.tile_pool(name="qk", bufs=2) as qk_pool,
            tc.tile_pool(name="vx", bufs=2) as vx_pool,
            tc.tile_pool(name="attn", bufs=10) as attn_pool,
            tc.tile_pool(name="yext", bufs=2) as yext_pool,
            tc.tile_pool(name="fin", bufs=6) as fin_pool,
            tc.tile_pool(name="psc", bufs=2, space="PSUM") as psum_sc,
            tc.tile_pool(name="psy", bufs=1, space="PSUM") as psum_y,
            tc.tile_pool(name="psp", bufs=2, space="PSUM") as psum_p,
        ):
            wx_sb = const_pool.tile([65, 65], F16, tag="wx")
            nc.sync.dma_start(wx_sb[:], wx_d[:])
            bb_sb = const_pool.tile([128, 64], F32, tag="bb")
            nc.sync.dma_start(bb_sb[:], bb_d[:])

            for h in range(hpc):
                qt_sb = qk_pool.tile([128, seq], F16, tag="qt")
                nc.sync.dma_start(qt_sb[:], qt_d[h])
                kt_sb = qk_pool.tile([128, half * 128], F16, tag="kt")
                nc.sync.dma_start(kt_sb[:], kt_d[h])
                vx_sb = vx_pool.tile([128, n_ch, 65], F16, tag="vx")
                nc.sync.dma_start(vx_sb[:], vx_d[h].rearrange("c p e -> p c e"))

                u = 0
                for m in range(n_m):
                    y_ps = psum_y.tile([65, QB], F32, tag="y")
                    for p in range(half):
                        kcols = slice(p * 128, (p + 1) * 128)
                        for jj in range(n_jj):
                            q0 = m * QB + jj * 512
                            sc_ps = psum_sc.tile([128, 1024], F32, tag="sc")
                            # chunk p on PE rows 0-63, chunk p+half on rows 64-127
                            nc.tensor.matmul(
                                sc_ps[:, 0:512],
                                kt_sb[0:64, kcols],
                                qt_sb[0:64, q0:q0 + 512],
                                start=True, stop=True,
                            )
                            nc.tensor.matmul(
                                sc_ps[:, 512:1024],
                                kt_sb[64:128, kcols],
                                qt_sb[64:128, q0:q0 + 512],
                                start=True, stop=True,
                            )
                            at_sb = attn_pool.tile([128, 1024], F16, tag="at")
                            if u % 8 in dve_slots:
                                nc.vector.tensor_scalar(
                                    at_sb[:].bitcast(mybir.dt.int16),
                                    sc_ps[:],
                                    EXPA, EXPB,
                                    op0=mybir.AluOpType.mult,
                                    op1=mybir.AluOpType.add,
                                )
                            else:
                                nc.scalar.activation(
                                    at_sb[:], sc_ps[:],
                                    mybir.ActivationFunctionType.Exp,
                                    bias=0.0, scale=SCALE,
                                )
                            u += 1
                            yslice = slice(jj * 512, jj * 512 + 512)
                            nc.tensor.matmul(
                                y_ps[:, yslice],
                                vx_sb[:, p],
                                at_sb[:, 0:512],
                                start=(p == 0), stop=False,
                            )
                            nc.tensor.matmul(
                                y_ps[:, yslice],
                                vx_sb[:, p + half],
                                at_sb[:, 512:1024],
                                start=False, stop=(p == half - 1),
                            )
                    y_sb = yext_pool.tile([65, QB], F16, tag="y16")
                    nc.vector.tensor_copy(y_sb[:], y_ps[:])
                    for t in range(n_qt):
                        p_ps = psum_p.tile([128, 65], F32, tag="p")
                        nc.tensor.matmul(
                            p_ps[:],
                            y_sb[:, t * 128:(t + 1) * 128],
                            wx_sb[:],
                            start=True, stop=True,
                        )
                        recip = fin_pool.tile([128, 1], F32, tag="recip")
                        nc.vector.reciprocal(recip[:], p_ps[:, 64:65])
                        o_sb = fin_pool.tile([128, 64], F32, tag="o")
                        nc.vector.scalar_tensor_tensor(
                            o_sb[:],
                            p_ps[:, 0:64],
                            recip[:],
                            bb_sb[:],
                            op0=mybir.AluOpType.mult,
                            op1=mybir.AluOpType.add,
                        )
                        r0 = m * QB + t * 128
                        nc.sync.dma_start(out_d[h, r0:r0 + 128, :], o_sb[:])
    nc.finalize()
    return nc


def shard_inputs(q, k, v, w_out, b_out, hpc=HPC, seq=S, n_cores=N_CORES):
    """Host-side layout prep: per-core fp16 transposed shards."""
    n_ch = seq // 128
    half = n_ch // 2
    nh = n_cores * hpc
    qT = np.asarray(q, dtype=np.float32).reshape(nh, seq, D).transpose(0, 2, 1)
    qT = qT.astype(np.float16)                      # [nh, 64, seq]
    qdup = np.concatenate([qT, qT], axis=1)         # [nh, 128, seq]
    kT = np.asarray(k, dtype=np.float32).reshape(nh, seq, D).transpose(0, 2, 1)
    kT = kT.astype(np.float16)                      # [nh, 64, seq]
    kpack = np.concatenate(                          # [nh, 128, half*128]
        [kT[:, :, :half * 128], kT[:, :, half * 128:]], axis=1
    )
    qdup = np.ascontiguousarray(qdup)
    kpack = np.ascontiguousarray(kpack)

    vf = np.asarray(v, dtype=np.float32).reshape(nh, seq, D)
    vx = np.ones((nh, seq, 65), dtype=np.float16)
    vx[:, :, :64] = vf
    vx = vx.reshape(nh, n_ch, 128, 65)

    wx = np.zeros((65, 65), dtype=np.float16)
    wx[:64, :64] = np.asarray(w_out, dtype=np.float32).T
    wx[64, 64] = 1.0
    bb = np.broadcast_to(
        np.asarray(b_out, dtype=np.float32)[None, :], (128, 64)
    ).copy()

    in_maps = []
    for c in range(n_cores):
        s0, s1 = c * hpc, (c + 1) * hpc
        in_maps.append({
            "qt": qdup[s0:s1],
            "kt": kpack[s0:s1],
            "vx": vx[s0:s1],
            "wx": wx,
            "bb": bb,
        })
    return in_maps


def kernel(q, k, v, w_out, b_out):
    global LAST_RESULT
    key = "full"
    if key not in _CACHED:
        _CACHED[key] = build_bass()
    nc = _CACHED[key]

    in_maps = shard_inputs(q, k, v, w_out, b_out)
    res = run_bass_kernel_spmd(
        nc, in_maps, core_ids=list(range(N_CORES)),
        trace=TRACE, **TRACE_KWARGS,
    )
    LAST_RESULT = res
    out = np.concatenate(
        [r["out"][None] for r in res.results], axis=0
    )  # [n_cores, hpc, S, 64]
    return out.reshape(B, H, S, 64)


# revision 13
# speedup vs baseline: 1.0034x; 1.0034x over previous
# Multi-head attention + output projection kernel for 8 TRN2 NeuronCores.
#
# Problem: q,k,v [4,16,2048,64] fp32; w_out [64,64]; b_out [64]
#   out = softmax(q @ k^T / sqrt(64)) @ v @ w_out^T + b_out
#
# Strategy:
#  - 64 (batch, head) pairs sharded 8-per-core (pure data parallel, no collectives).
#  - Host-side layout prep: Q^T duplicated on both partition halves, K^T packed so
#    chunk c and c+8 sit in partition rows 0-63 / 64-127 (fp16, d on partitions),
#    V with a ones-column appended (softmax denominator rides the PE matmul),
#    w_out^T padded to [65,65] with a 1 in the corner (row-sum rides the projection).
#  - Scores are computed transposed [k_chunk, q] with the contract dim d=64 split
#    across PE row groups: two chunk matmuls run concurrently via tile_position
#    row packing, doubling QK^T throughput.
#  - exp on ScalarE straight out of PSUM (no max-subtraction: |scores/8| < ~7,
#    fp16-safe), writing the attn^T tile fp16 to SBUF.
#  - y_ext^T = V_ext^T @ attn^T accumulated in PSUM over k chunks; projection
#    returns to [q, 65] orientation; normalize+bias are per-partition VectorE ops.

import math

import numpy as np

import concourse.bass as bass
import concourse.mybir as mybir
import concourse.tile as tile
from concourse import bacc
from concourse.bass_utils import run_bass_kernel_spmd

F16 = mybir.dt.float16
F32 = mybir.dt.float32

B, H, S, D = 4, 16, 2048, 64
N_CORES = 8
N_HEADS = B * H                    # 64
HPC = N_HEADS // N_CORES           # 8 heads per core
SCALE = 1.0 / math.sqrt(D)         # 1/8

# fp16 Schraudolph exp for the VectorE path: exp(s/8) ~= bitcast_f16(
# int16_trunc(s * EXPA + EXPB)). Constant calibrated for truncation
# (DVE fp32->int16 converts toward zero); max rel err ~3.0%, which the
# softmax normalization and 2048-term averaging wash down to ~6e-3 end-to-end.
EXPA = 1024.0 / (8.0 * math.log(2.0))
EXPB = 15315.5    # calibrated for round-to-nearest (HW convert behavior)

# test.py can flip these before calling kernel() to capture a profile.
TRACE = False
TRACE_KWARGS = {}
LAST_RESULT = None

_CACHED = {}


def build_bass(hpc=HPC, seq=S, dve_slots=(0, 3, 6)):
    """Build the per-core Bass program. Parameterized so a small config can be
    simulated in CoreSim. Requires seq % 256 == 0.

    dve_slots: which unit indices (mod 8) compute exp on VectorE via the
    Schraudolph approximation instead of ScalarE, balancing the two engines."""
    QB = min(512, seq)             # q columns per y-accumulation block
    n_m = seq // QB                # y blocks per head
    n_ch = seq // 128              # k chunks per head
    half = n_ch // 2               # chunk pairs per head
    n_qt = QB // 128               # 128-row q tiles per block

    nc = bacc.Bacc("TRN2", target_bir_lowering=False, debug=False)

    qt_d = nc.dram_tensor("qt", [hpc, 128, seq], F16, kind="ExternalInput").ap()
    kt_d = nc.dram_tensor("kt", [hpc, 128, half * 128], F16, kind="ExternalInput").ap()
    vx_d = nc.dram_tensor("vx", [hpc, n_ch, 128, 65], F16, kind="ExternalInput").ap()
    wx_d = nc.dram_tensor("wx", [65, 65], F16, kind="ExternalInput").ap()
    bb_d = nc.dram_tensor("bb", [128, 64], F32, kind="ExternalInput").ap()
    out_d = nc.dram_tensor("out", [hpc, seq, 64], F32, kind="ExternalOutput").ap()

    with tile.TileContext(nc) as tc:
        with (
            tc.tile_pool(name="const", bufs=1) as const_pool,
            tc.tile_pool(name="qk", bufs=2) as qk_pool,
            tc.tile_pool(name="vx", bufs=2) as vx_pool,
            tc.tile_pool(name="attn", bufs=10) as attn_pool,
            tc.tile_pool(name="yext", bufs=2) as yext_pool,
            tc.tile_pool(name="fin", bufs=6) as fin_pool,
            tc.tile_pool(name="psc", bufs=2, space="PSUM") as psum_sc,
            tc.tile_pool(name="psy", bufs=2, space="PSUM") as psum_y,
            tc.tile_pool(name="psp", bufs=2, space="PSUM") as psum_p,
        ):
            wx_sb = const_pool.tile([65, 65], F16, tag="wx")
            nc.sync.dma_start(wx_sb[:], wx_d[:])
            bb_sb = const_pool.tile([128, 64], F32, tag="bb")
            nc.sync.dma_start(bb_sb[:], bb_d[:])

            for h in range(hpc):
                qt_sb = qk_pool.tile([128, seq], F16, tag="qt")
                nc.sync.dma_start(qt_sb[:], qt_d[h])
                kt_sb = qk_pool.tile([128, half * 128], F16, tag="kt")
                nc.sync.dma_start(kt_sb[:], kt_d[h])
                vx_sb = vx_pool.tile([128, n_ch, 65], F16, tag="vx")
                nc.sync.dma_start(vx_sb[:], vx_d[h].rearrange("c p e -> p c e"))

                u = 0
                for m in range(n_m):
                    y_ps = psum_y.tile([65, QB], F32, tag="y")
                    q0 = m * QB
                    for p in range(half):
                        kcols = slice(p * 128, (p + 1) * 128)
                        sc_ps = psum_sc.tile([128, 1024], F32, tag="sc")
                        # chunk p on PE rows 0-63, chunk p+half on rows 64-127
                        nc.tensor.matmul(
                            sc_ps[:, 0:512],
                            kt_sb[0:64, kcols],
                            qt_sb[0:64, q0:q0 + 512],
                            start=True, stop=True,
                        )
                        nc.tensor.matmul(
                            sc_ps[:, 512:1024],
                            kt_sb[64:128, kcols],
                            qt_sb[64:128, q0:q0 + 512],
                            start=True, stop=True,
                        )
                        at_sb = attn_pool.tile([128, 1024], F16, tag="at")
                        if u % 8 in dve_slots:
                            nc.vector.tensor_scalar(
                                at_sb[:].bitcast(mybir.dt.int16),
                                sc_ps[:],
                                EXPA, EXPB,
                                op0=mybir.AluOpType.mult,
                                op1=mybir.AluOpType.add,
                            )
                        else:
                            nc.scalar.activation(
                                at_sb[:], sc_ps[:],
                                mybir.ActivationFunctionType.Exp,
                                bias=0.0, scale=SCALE,
                            )
                        u += 1
                        nc.tensor.matmul(
                            y_ps[:],
                            vx_sb[:, p],
                            at_sb[:, 0:512],
                            start=(p == 0), stop=False,
                        )
                        nc.tensor.matmul(
                            y_ps[:],
                            vx_sb[:, p + half],
                            at_sb[:, 512:1024],
                            start=False, stop=(p == half - 1),
                        )
                    y_sb = yext_pool.tile([65, QB], F16, tag="y16")
                    nc.vector.tensor_copy(y_sb[:], y_ps[:])
                    for t in range(n_qt):
                        p_ps = psum_p.tile([128, 65], F32, tag="p")
                        nc.tensor.matmul(
                            p_ps[:],
                            y_sb[:, t * 128:(t + 1) * 128],
                            wx_sb[:],
                            start=True, stop=True,
                        )
                        recip = fin_pool.tile([128, 1], F32, tag="recip")
                        nc.vector.reciprocal(recip[:], p_ps[:, 64:65])
                        o_sb = fin_pool.tile([128, 64], F32, tag="o")
                        nc.vector.scalar_tensor_tensor(
                            o_sb[:],
                            p_ps[:, 0:64],
                            recip[:],
                            bb_sb[:],
                            op0=mybir.AluOpType.mult,
                            op1=mybir.AluOpType.add,
                        )
                        r0 = m * QB + t * 128
                        nc.sync.dma_start(out_d[h, r0:r0 + 128, :], o_sb[:])
    nc.finalize()
    return nc


def shard_inputs(q, k, v, w_out, b_out, hpc=HPC, seq=S, n_cores=N_CORES):
    """Host-side layout prep: per-core fp16 transposed shards."""
    n_ch = seq // 128
    half = n_ch // 2
    nh = n_cores * hpc
    qT = np.asarray(q, dtype=np.float32).reshape(nh, seq, D).transpose(0, 2, 1)
    qT = qT.astype(np.float16)                      # [nh, 64, seq]
    qdup = np.concatenate([qT, qT], axis=1)         # [nh, 128, seq]
    kT = np.asarray(k, dtype=np.float32).reshape(nh, seq, D).transpose(0, 2, 1)
    kT = kT.astype(np.float16)                      # [nh, 64, seq]
    kpack = np.concatenate(                          # [nh, 128, half*128]
        [kT[:, :, :half * 128], kT[:, :, half * 128:]], axis=1
    )
    qdup = np.ascontiguousarray(qdup)
    kpack = np.ascontiguousarray(kpack)

    vf = np.asarray(v, dtype=np.float32).reshape(nh, seq, D)
    vx = np.ones((nh, seq, 65), dtype=np.float16)
    vx[:, :, :64] = vf
    vx = vx.reshape(nh, n_ch, 128, 65)

    wx = np.zeros((65, 65), dtype=np.float16)
    wx[:64, :64] = np.asarray(w_out, dtype=np.float32).T
    wx[64, 64] = 1.0
    bb = np.broadcast_to(
        np.asarray(b_out, dtype=np.float32)[None, :], (128, 64)
    ).copy()

    in_maps = []
    for c in range(n_cores):
        s0, s1 = c * hpc, (c + 1) * hpc
        in_maps.append({
            "qt": qdup[s0:s1],
            "kt": kpack[s0:s1],
            "vx": vx[s0:s1],
            "wx": wx,
            "bb": bb,
        })
    return in_maps


def kernel(q, k, v, w_out, b_out):
    global LAST_RESULT
    key = "full"
    if key not in _CACHED:
        _CACHED[key] = build_bass()
    nc = _CACHED[key]

    in_maps = shard_inputs(q, k, v, w_out, b_out)
    res = run_bass_kernel_spmd(
        nc, in_maps, core_ids=list(range(N_CORES)),
        trace=TRACE, **TRACE_KWARGS,
    )
    LAST_RESULT = res
    out = np.concatenate(
        [r["out"][None] for r in res.results], axis=0
    )  # [n_cores, hpc, S, 64]
    return out.reshape(B, H, S, 64)
